# revision 20
# baseline (speedup 1.0000x reference)
"""Fused multi-head attention (QKV proj + RMSNorm + RoPE + softmax attention +
output proj) for Trainium2, sharded over 8 NeuronCores as batch x head-groups.

Sharding: core c handles batch b = c // 4 and heads 4*(c%4) .. 4*(c%4)+3.
Each core computes a partial output [S, D] (its head-group's contribution via
its slice of Wo); the host sums the 4 partials per batch element and adds bo.
"""
import sys
sys.path.insert(0, "/opt/trn_rl_repo")
import os
import numpy as np

import concourse.bass as bass
import concourse.tile as tile
from concourse.tile import add_dep_helper
from concourse import bacc, mybir
from concourse.bass_utils import run_bass_kernel_spmd

f32 = mybir.dt.float32
f32r = mybir.dt.float32r
bf16 = mybir.dt.bfloat16
AF = mybir.ActivationFunctionType

DIM = 1024
NUM_HEADS = 16
HD = 64
B, S = 2, 2048
EPS = 1e-6
NCORES = 8
GROUPS = 4                 # head-groups (cores per batch element)
E = DIM // GROUPS          # 256 output dims per core (4 heads)
NK = DIM // 128            # 8 contraction k-tiles for projections

LAST_EXEC_NS = None


def _psl(t, start, step, count):
    """Partition-sliced AP: rows start, start+step, ... (count rows)."""
    return bass.AP(tensor=t.tensor, offset=t.offset + start * t.ap[0][0],
                   ap=[[step * t.ap[0][0], count]] + list(t.ap[1:]))


def _bcast_rows(t, row, nrows):
    """AP reading partition `row` of t, replicated nrows times."""
    return bass.AP(tensor=t.tensor, offset=t.offset + row * t.ap[0][0],
                   ap=[[0, nrows]] + list(t.ap[1:]))


def build_program(with_bias: bool, dbg: bool = False):
    nk = NK + 1 if with_bias else NK
    kdim = nk * 128
    nc = bacc.Bacc("TRN2", target_bir_lowering=False, debug=False,
                   enable_asserts=False, num_devices=NCORES)

    xT = nc.dram_tensor("xT", [kdim, S], f32r, kind="ExternalInput").ap()
    wqT = nc.dram_tensor("wqT", [kdim, E], f32r, kind="ExternalInput").ap()
    wkT = nc.dram_tensor("wkT", [kdim, E], f32r, kind="ExternalInput").ap()
    wvT = nc.dram_tensor("wvT", [kdim, E], f32r, kind="ExternalInput").ap()
    woT = nc.dram_tensor("woT", [E, DIM], f32r, kind="ExternalInput").ap()
    cosq = nc.dram_tensor("cosq", [128, S], f32, kind="ExternalInput").ap()
    sinq = nc.dram_tensor("sinq", [128, S], f32, kind="ExternalInput").ap()
    cosk = nc.dram_tensor("cosk", [128, S], f32, kind="ExternalInput").ap()
    sink = nc.dram_tensor("sink", [128, S], f32, kind="ExternalInput").ap()
    out = nc.dram_tensor("out", [S, DIM], f32, kind="ExternalOutput").ap()
    dbg_t = {}
    if dbg:
        for nm, shape in (("d_qTr", [2, 128, S]), ("d_kTr", [2, 128, S]),
                          ("d_v", [16, 128, E]),
                          ("d_rkT", [4, 128, 16]), ("d_rd", [4, 128, 512]),
                          ("d_oTn", [2, 4, 128, 512]),
                          ("d_exp", [128, 512]), ("d_sc", [128, 512])):
            dbg_t[nm] = nc.dram_tensor(nm, shape, f32, kind="ExternalOutput").ap()

    with tile.TileContext(nc) as tc:
        _emit(tc, nc, nk, xT, wqT, wkT, wvT, woT, cosq, sinq, cosk, sink, out,
              dbg_t)
    nc.compile()
    return nc


def _emit(tc, nc, nk, xT, wqT, wkT, wvT, woT, cosq, sinq, cosk, sink, out,
          dbg_t=None):
    dbg_t = dbg_t or {}
    from contextlib import ExitStack
    NSB = 4            # 512-wide s-blocks
    NST = 16           # 128-wide s-tiles

    persist = tc.alloc_tile_pool(name="persist", bufs=1)
    dscratch = tc.alloc_tile_pool(name="dscratch", bufs=3, space="DRAM")
    # roped q/k, transposed layout [e(=2 heads x 64), s] per e-tile
    qT_rope = [persist.tile([128, S], f32r, name=f"qTr{e}") for e in range(2)]
    kT_rope = [persist.tile([128, S], f32r, name=f"kTr{e}") for e in range(2)]
    # v in [s, e] layout per s-tile, bf16
    v_sb = [persist.tile([128, E], bf16, name=f"vsb{st}") for st in range(NST)]
    # per-head rstd_k in transposed layout [s_within_tile, s_tile]
    rstd_kT = [persist.tile([128, NST], f32, name=f"rkT{h}") for h in range(4)]
    ones_b = persist.tile([128, 1], bf16, name="ones_b")

    ones_f = persist.tile([128, 1], f32, name="ones_f")
    nc.vector.memset(ones_f[:], 1.0)
    nc.vector.tensor_copy(ones_b[:], ones_f[:])
    ones2_f = persist.tile([128, 33], f32, name="ones2_f")
    nc.vector.memset(ones2_f[:], 0.0)
    nc.vector.memset(ones2_f[0:64, 0:1], 1.0)
    nc.vector.memset(ones2_f[64:128, 32:33], 1.0)
    ones2_b = persist.tile([128, 33], bf16, name="ones2_b")
    nc.vector.tensor_copy(ones2_b[:], ones2_f[:])
    eps_t = persist.tile([128, 1], f32, name="eps_t")
    nc.vector.memset(eps_t[:], EPS)

    # ---------------- Stage A: projections + norm + rope ----------------
    with ExitStack() as stA:
        consts = stA.enter_context(tc.tile_pool(name="constsA", bufs=1))
        xt = []
        for k in range(nk):
            t = consts.tile([128, S], f32r, name=f"xt{k}")
            nc.sync.dma_start(t[:], xT[k * 128:(k + 1) * 128, :])
            xt.append(t)
        wq, wk, wv = [], [], []
        for nm, dram, lst in (("wq", wqT, wq), ("wk", wkT, wk), ("wv", wvT, wv)):
            for k in range(nk):
                t = consts.tile([128, E], f32r, name=f"{nm}{k}")
                nc.sync.dma_start(t[:], dram[k * 128:(k + 1) * 128, :])
                lst.append(t)

        ropes = stA.enter_context(tc.tile_pool(name="ropesA", bufs=2))
        temps = stA.enter_context(tc.tile_pool(name="tempsA", bufs=2))
        raws = stA.enter_context(tc.tile_pool(name="rawsA", bufs=3))
        psA = stA.enter_context(tc.tile_pool(name="psA", bufs=2, space="PSUM"))
        psSq = stA.enter_context(tc.tile_pool(name="psSq", bufs=2, space="PSUM"))

        # ---- q and k transposed projections + norm + rope ----
        for tname, wlist, cos_d, sin_d, dest, is_q in (
                ("q", wq, cosq, sinq, qT_rope, True),
                ("k", wk, cosk, sink, kT_rope, False)):
            # k-side packed sum-of-squares psum tile: [128, 4 heads * 16 stiles]
            if not is_q:
                sqT_pack = psSq.tile([128, 64], f32, name="sqT_pack", bufs=1)
                ksq_first = [True]
            for sb in range(NSB):
                ssl = slice(sb * 512, (sb + 1) * 512)
                lnq_h = [None] * 4
                raw_e = []
                for e in range(2):
                    proj_ps = psA.tile([128, 512], f32, name="proj_ps")
                    for k in range(nk):
                        nc.tensor.matmul(proj_ps[:], wlist[k][:, e * 128:(e + 1) * 128],
                                         xt[k][:, ssl], start=(k == 0),
                                         stop=(k == nk - 1))
                    raw = raws.tile([128, 512], f32, name="raw")
                    nc.vector.tensor_copy(raw[:], proj_ps[:])
                    raw_e.append(raw)
                    sq = temps.tile([128, 512], bf16, name="sq")
                    nc.vector.tensor_mul(sq[:], proj_ps[:], raw[:])
                    if is_q:
                        # block-diagonal ones: per-head sums at psum rows 0/1
                        sumsq2 = psSq.tile([33, 512], f32, name="sumsq2")
                        nc.tensor.matmul(sumsq2[:], ones2_b[:], sq[:],
                                         start=True, stop=True)
                        for hl in range(2):
                            hg = 2 * e + hl
                            lh = temps.tile([1, 512], f32, name=f"lnq{hg}")
                            nc.scalar.activation(lh[0:1, :],
                                                 sumsq2[32 * hl:32 * hl + 1, :],
                                                 AF.Ln, bias=eps_t[0:1, :],
                                                 scale=1.0 / HD)
                            lnq_h[hg] = lh
                    for hl in range(2):
                        hg = 2 * e + hl
                        if is_q:
                            pass
                        else:
                            # transposed sumsq: out [s_within, stile] col hg*16+st
                            for stl in range(4):
                                st = 4 * sb + stl
                                col = hg * NST + st
                                first = ksq_first[0]
                                ksq_first[0] = False
                                nc.tensor.matmul(
                                    sqT_pack[:, col:col + 1],
                                    sq[64 * hl:64 * hl + 64,
                                       stl * 128:(stl + 1) * 128],
                                    ones_b[64 * hl:64 * hl + 64, :],
                                    start=first, stop=(col == 63 and not is_q
                                                       and hg == 3 and st == 15),
                                    tile_position=(64 * hl, 0))
                for e in range(2):
                    if is_q:
                        rqb = temps.tile([128, 512], f32, name="rqb")
                        for hl in range(2):
                            scr = dscratch.tile([1, 512], f32, name="scr")
                            nc.sync.dma_start(scr[:],
                                              lnq_h[2 * e + hl][0:1, :])
                            nc.gpsimd.dma_start(
                                rqb[64 * hl:64 * hl + 64, :],
                                _bcast_rows(scr, 0, 64))
                        nc.scalar.activation(rqb[:], rqb[:], AF.Exp, scale=-0.5)
                        src = temps.tile([128, 512], f32, name="src")
                        nc.vector.tensor_mul(src[:], raw_e[e][:], rqb[:])
                    else:
                        src = raw_e[e]
                    cos_t = ropes.tile([128, 512], f32, name="cos_t")
                    nc.sync.dma_start(cos_t[:], cos_d[:, ssl])
                    sin_t = ropes.tile([128, 512], f32, name="sin_t")
                    nc.sync.dma_start(sin_t[:], sin_d[:, ssl])
                    swp = temps.tile([128, 512], f32, name="swp")
                    for blk in range(4):
                        lo, hi = 64 * (blk // 2), 32 * (blk % 2)
                        a = lo + hi
                        bsl = lo + 32 - hi
                        nc.sync.dma_start(swp[a:a + 32, :], src[bsl:bsl + 32, :])
                    t1 = temps.tile([128, 512], f32, name="t1")
                    nc.vector.tensor_mul(t1[:], src[:], cos_t[:])
                    t2 = temps.tile([128, 512], f32, name="t2")
                    nc.vector.tensor_mul(t2[:], swp[:], sin_t[:])
                    nc.vector.tensor_add(dest[e][:, ssl], t1[:], t2[:])

        # k rstd: ln+exp full-lane on packed [128,16] blocks
        for h in range(4):
            nc.scalar.activation(rstd_kT[h][:], sqT_pack[:, h * NST:(h + 1) * NST],
                                 AF.Ln, bias=eps_t[:], scale=1.0 / HD)
            nc.scalar.activation(rstd_kT[h][:], rstd_kT[h][:], AF.Exp, scale=-0.5)

        # ---- v projection ([s, e] layout) ----
        for st in range(NST):
            vps = psA.tile([128, E], f32, name="vps")
            for k in range(nk):
                nc.tensor.matmul(vps[:], xt[k][:, st * 128:(st + 1) * 128],
                                 wv[k][:], start=(k == 0), stop=(k == nk - 1))
            nc.vector.tensor_copy(v_sb[st][:], vps[:])

    if dbg_t:
        for e in range(2):
            nc.sync.dma_start(dbg_t["d_qTr"][e], qT_rope[e][:].bitcast(f32))
            nc.sync.dma_start(dbg_t["d_kTr"][e], kT_rope[e][:].bitcast(f32))
        for h in range(4):
            nc.sync.dma_start(dbg_t["d_rkT"][h], rstd_kT[h][:])

    # ---------------- Stage B: attention ----------------
    late = tc.alloc_tile_pool(name="late", bufs=1)
    wo_sb = []
    for e in range(2):
        t = late.tile([128, DIM], f32r, name=f"wo{e}")
        nc.sync.dma_start(t[:], woT[e * 128:(e + 1) * 128, :])
        wo_sb.append(t)
    oTn = [[None] * NSB, [None] * NSB]

    with ExitStack() as stB:
        exps = stB.enter_context(tc.tile_pool(name="expsB", bufs=3))
        tempsB = stB.enter_context(tc.tile_pool(name="tempsB", bufs=2))
        psS = stB.enter_context(tc.tile_pool(name="psS", bufs=2, space="PSUM"))
        psAV = stB.enter_context(tc.tile_pool(name="psAV", bufs=1, space="PSUM"))

        for qb in range(NSB):
            qsl = slice(qb * 512, (qb + 1) * 512)
            avp = [psAV.tile([128, 512], f32, name=f"avp{e}", bufs=1)
                   for e in range(2)]
            den_ps = psAV.tile([128, 512], f32, name="den_ps", bufs=1)
            for sk in range(NST):
                ksl = slice(sk * 128, (sk + 1) * 128)
                for e in range(2):
                    exp_eh = []
                    for hl in range(2):
                        hg = 2 * e + hl
                        hsl = slice(64 * hl, 64 * hl + 64)
                        sc = psS.tile([128, 512], f32, name=f"sc{hl}")
                        nc.tensor.matmul(sc[:], kT_rope[e][hsl, ksl],
                                         qT_rope[e][hsl, qsl],
                                         start=True, stop=True,
                                         tile_position=(64 * hl, 0))
                        ex = exps.tile([128, 512], bf16, name=f"ex{hl}")
                        if dbg_t and qb == 0 and sk == 0 and e == 0 and hl == 0:
                            scf = exps.tile([128, 512], f32, name="scf")
                            nc.vector.tensor_copy(scf[:], sc[:])
                            nc.sync.dma_start(dbg_t["d_sc"], scf[:])
                        nc.scalar.activation(ex[:], sc[:], AF.Exp,
                                             scale=rstd_kT[hg][:, sk:sk + 1])
                        if dbg_t and qb == 0 and sk == 0 and e == 0 and hl == 0:
                            exf = exps.tile([128, 512], f32, name="exf")
                            nc.vector.tensor_copy(exf[:], ex[:])
                            nc.sync.dma_start(dbg_t["d_exp"], exf[:])
                        exp_eh.append(ex)
                    for hl in range(2):
                        hg = 2 * e + hl
                        nc.tensor.matmul(
                            avp[e][64 * hl:64 * hl + 64, :],
                            v_sb[sk][:, e * 128 + 64 * hl:e * 128 + 64 * hl + 64],
                            exp_eh[hl][:],
                            start=(sk == 0), stop=(sk == NST - 1),
                            tile_position=(0, 64 * hl), skip_group_check=True)
                        nc.tensor.matmul(
                            den_ps[32 * hg:32 * hg + 1, :],
                            ones_b[:], exp_eh[hl][:],
                            start=(sk == 0), stop=(sk == NST - 1),
                            tile_position=(0, 32 * hg), skip_group_check=True)
            # normalize: reciprocal cross-base to partition-0 tiles
            rd_h = []
            for hg in range(4):
                rh = tempsB.tile([1, 512], f32, name=f"rd{hg}")
                nc.vector.reciprocal(rh[0:1, :],
                                     den_ps[32 * hg:32 * hg + 1, :])
                rd_h.append(rh)
            if dbg_t:
                for hg in range(4):
                    nc.sync.dma_start(dbg_t["d_rd"][qb][32 * hg:32 * hg + 1, :],
                                      rd_h[hg][0:1, :])
            for e in range(2):
                rdb = tempsB.tile([128, 512], f32, name="rdb")
                for hl in range(2):
                    scr2 = dscratch.tile([1, 512], f32, name="scr2")
                    nc.sync.dma_start(scr2[:], rd_h[2 * e + hl][0:1, :])
                    nc.gpsimd.dma_start(rdb[64 * hl:64 * hl + 64, :],
                                        _bcast_rows(scr2, 0, 64))
                ot = late.tile([128, 512], f32r, name=f"oTn{e}_{qb}")
                nc.vector.tensor_mul(ot[:], avp[e][:], rdb[:])
                if dbg_t:
                    nc.sync.dma_start(dbg_t["d_oTn"][e, qb], ot[:].bitcast(f32))
                oTn[e][qb] = ot

    # ---------------- Stage C: output projection ----------------
    with ExitStack() as stC:
        outs = stC.enter_context(tc.tile_pool(name="outsC", bufs=3))
        psO = stC.enter_context(tc.tile_pool(name="psO", bufs=2, space="PSUM"))
        for st in range(NST):
            qb, sub = st // 4, st % 4
            ot_sb = outs.tile([128, DIM], f32, name="ot_sb")
            for eh in range(2):
                osl = slice(eh * 512, (eh + 1) * 512)
                ops_ = psO.tile([128, 512], f32, name="ops")
                for e in range(2):
                    nc.tensor.matmul(ops_[:],
                                     oTn[e][qb][:, sub * 128:(sub + 1) * 128],
                                     wo_sb[e][:, osl],
                                     start=(e == 0), stop=(e == 1))
                if eh == 0:
                    nc.vector.tensor_copy(ot_sb[:, osl], ops_[:])
                else:
                    nc.scalar.copy(ot_sb[:, osl], ops_[:])
            nc.sync.dma_start(out[st * 128:(st + 1) * 128, :], ot_sb[:])

    late.release()
    dscratch.release()
    persist.release()


_PROGRAM_CACHE = {}


def _get_program(with_bias, dbg=False):
    key = (bool(with_bias), dbg)
    if key not in _PROGRAM_CACHE:
        _PROGRAM_CACHE[key] = build_program(with_bias, dbg)
    return _PROGRAM_CACHE[key]


# rows of q/k are de-interleaved per head: [re_0..re_31, im_0..im_31]
_DEINT = np.concatenate([np.arange(0, HD, 2), np.arange(1, HD, 2)])


def _rope_tables(cos_b, sin_b, norm_w, scale):
    """Build [128, S] cos/sin multiplier tables for the de-interleaved
    transposed rope layout (rows [evens | odds] per 64-row head block).

    out = src * cosT + block_swap(src) * sinT
    cos_b/sin_b: [S, HD//2]; norm_w: [HD]; returns (cosT, sinT) fp32 [128, S].
    """
    c32 = cos_b.T.astype(np.float32)               # [32, S]
    s32 = sin_b.T.astype(np.float32)
    c64 = np.concatenate([c32, c32], axis=0)       # same c_j for re and im rows
    s64 = np.concatenate([-s32, s32], axis=0)      # -s_j on re rows, +s_j on im
    w = norm_w.astype(np.float32)[_DEINT]          # de-interleaved norm weights
    wsw = np.concatenate([w[32:], w[:32]])         # block-swapped weights
    cosT = np.tile(c64 * w[:, None] * scale, (2, 1))
    sinT = np.tile(s64 * wsw[:, None] * scale, (2, 1))
    return np.ascontiguousarray(cosT, np.float32), np.ascontiguousarray(sinT, np.float32)


def kernel(hidden_states, rope_cos, rope_sin, Wq, bq, Wk, bk, Wv, bv,
           q_norm_w, k_norm_w, Wo, bo):
    global LAST_EXEC_NS
    hidden_states = np.asarray(hidden_states, np.float32)
    rope_cos = np.asarray(rope_cos, np.float32)
    rope_sin = np.asarray(rope_sin, np.float32)
    Wq, Wk, Wv, Wo = (np.asarray(a, np.float32) for a in (Wq, Wk, Wv, Wo))
    bq, bk, bv, bo = (np.asarray(a, np.float32) for a in (bq, bk, bv, bo))
    q_norm_w = np.asarray(q_norm_w, np.float32)
    k_norm_w = np.asarray(k_norm_w, np.float32)

    with_bias = bool(np.any(bq) or np.any(bk) or np.any(bv))
    nc = _get_program(with_bias)

    in_maps = []
    xTs, cosqs, sinqs, cosks, sinks = {}, {}, {}, {}, {}
    for b in range(B):
        xT = np.ascontiguousarray(hidden_states[b].T)          # [D, S]
        if with_bias:
            aug = np.zeros((128, S), np.float32)
            aug[0] = 1.0
            xT = np.concatenate([xT, aug], axis=0)
        xTs[b] = xT
        cosqs[b], sinqs[b] = _rope_tables(rope_cos[b], rope_sin[b], q_norm_w, 1.0)
        cosks[b], sinks[b] = _rope_tables(rope_cos[b], rope_sin[b], k_norm_w,
                                          1.0 / np.sqrt(HD))

    def wslice(W, bias, g, deint):
        rows = np.arange(g * E, (g + 1) * E)
        if deint:
            rows = rows.reshape(GROUPS, HD)[:, _DEINT].ravel()
        wT = np.ascontiguousarray(W[rows, :].T)                # [D, E]
        if with_bias:
            aug = np.zeros((128, E), np.float32)
            aug[0] = bias[rows]
            wT = np.concatenate([wT, aug], axis=0)
        return wT

    for c in range(NCORES):
        b, g = c // GROUPS, c % GROUPS
        in_maps.append({
            "xT": xTs[b],
            "wqT": wslice(Wq, bq, g, True),
            "wkT": wslice(Wk, bk, g, True),
            "wvT": wslice(Wv, bv, g, False),
            "woT": np.ascontiguousarray(Wo[:, g * E:(g + 1) * E].T),
            "cosq": cosqs[b], "sinq": sinqs[b],
            "cosk": cosks[b], "sink": sinks[b],
        })

    trace = os.environ.get("KERNEL_TRACE", "") == "1"
    try:
        res = run_bass_kernel_spmd(nc, in_maps, core_ids=list(range(NCORES)),
                                   trace=trace)
    except ModuleNotFoundError:
        res = run_bass_kernel_spmd(nc, in_maps, core_ids=list(range(NCORES)))
    LAST_EXEC_NS = res.exec_time_ns

    out = np.zeros((B, S, DIM), np.float32)
    for c in range(NCORES):
        b = c // GROUPS
        out[b] += res.results[c]["out"]
    out += bo
    return out
